# revision 30
# baseline (speedup 1.0000x reference)
"""nn_LEAStereo cost-volume + 3D-conv + bilinear upsample on 8 TRN2 NeuronCores.

Shapes (hardcoded per spec): x_feat/y_feat [2,3,32,88,116] f32,
w_match [1,64,3,3,3] f32. Output [2,33,260,346] f32.

Algorithm: only frame t=2 of each batch survives the [:, 2:] frame drop, so
2 frames matter. Contracting channels first (tap images Xt = wl^T x,
Yt = wr^T y, 27 taps each, both halves in one K=64 block-diagonal GEMM)
collapses the masked/shifted cost volume + 3x3x3 conv to 2D structure:

  cost[d,h,w] = F[h,w]*[w-d>=2] + G[w-d][h,w]  (w-d in -2..1)   (left half)
              + R[h,w-d] - right-edge corrections                (right half)

F/G/R are small images built from shifted sums of the tap images (batched
3-wide over kd: taps kd*9+kh*3+kw are 9 apart in the tap-major layout).

Sharding: 8 cores = 2 frames x 4 row-quarters of the 260 output rows. A
quarter's 65 output rows only touch ~24 input rows (bilinear support), so
each core computes just a 32-grid-row window. With only 24 rows on the
partition axis, the 33 disparities are packed 4 groups x 32 partitions
(d = 9*dg + jd), shrinking every batched assembly op 4x; the images are
broadcast to the 4 groups with one SBUF copy per group. Bilinear resize =
row-interp matmul (24->65 rows) + exact x3 column upsample; the three
column phases (weights 0, 1/3, 2/3) stay contiguous on-chip and are
interleaved on the host.
"""
import os
import numpy as np
import ml_dtypes

BF16 = ml_dtypes.bfloat16

C = 32
D = 33
DP = 36                    # padded disparity count: 4 groups x 9
H, W = 88, 116
WP = 118                   # grid row width (w+1 padded)
WB = 124                   # tap-block width: grid col w' lives at w'+3
GR = 32                    # grid rows per core window
RG = 16                    # rows per phase-A row-group (2 groups)
HC = 24                    # cost rows per core
GFLAT = GR * WP            # 3776
OH, OW = 260, 346
OHC = OH // 4              # 65 output rows per core
NTAP = 27
TWB = NTAP * WB            # 3348
H0S = (0, 21, 43, 65)      # per-quarter first needed input row

_BUILT = {}


def _row_matrix():
    ys = np.linspace(0.0, H - 1.0, OH)
    y0 = np.floor(ys).astype(np.int32)
    y1 = np.minimum(y0 + 1, H - 1)
    wy = (ys - y0).astype(np.float32)
    Rt = np.zeros((H, OH), dtype=np.float32)
    for j in range(OH):
        Rt[y0[j], j] += 1.0 - wy[j]
        Rt[y1[j], j] += wy[j]
    return Rt


def _build_nc():
    import concourse.bacc as bacc
    import concourse.mybir as mybir
    import bass_rust
    from concourse.tile import TileContext

    dt = mybir.dt
    Alu = mybir.AluOpType
    ActF = mybir.ActivationFunctionType

    nc = bacc.Bacc("TRN2", target_bir_lowering=False, debug=False)

    xp = nc.dram_tensor("xp", [C, GFLAT], dt.bfloat16, kind="ExternalInput")
    yp = nc.dram_tensor("yp", [C, GFLAT], dt.bfloat16, kind="ExternalInput")
    wc = nc.dram_tensor("wc", [2 * C, 64], dt.bfloat16, kind="ExternalInput")
    rt = nc.dram_tensor("rt", [128, OHC], dt.bfloat16, kind="ExternalInput")
    # phase-major outputs; host interleaves columns and upcasts p1/p2
    out0 = nc.dram_tensor("out0", [D, OHC, W], dt.float32,
                          kind="ExternalOutput")
    out12 = nc.dram_tensor("out12", [2, D, OHC, W], dt.bfloat16,
                           kind="ExternalOutput")
    # h-major tap images in DRAM [32, 27*124]
    xtd = nc.dram_tensor("xtd", [GR, TWB], dt.bfloat16)
    ytd = nc.dram_tensor("ytd", [GR, TWB], dt.bfloat16)

    def strided(tile_ap, offset, dims):
        """Custom (possibly overlapping) AP on a 2-D SBUF tile slice."""
        a = tile_ap.copy()
        a.ap = bass_rust.VecI64Pair([tuple(a.ap[0])] + list(dims))
        a.offset = a.offset + offset
        return a

    with TileContext(nc) as tc:
        with (
            tc.tile_pool(name="io", bufs=1) as io,
            tc.tile_pool(name="psA", bufs=1, space="PSUM") as psA,
            tc.tile_pool(name="stage", bufs=1) as stage,
            tc.tile_pool(name="psR", bufs=4, space="PSUM") as psR,
        ):
            A = nc.scalar     # ACT engine (+ 2nd HWDGE DMA ring)
            V = nc.vector
            G_ = nc.gpsimd
            S = nc.sync

            wcs = io.tile([2 * C, 64], dt.bfloat16)
            rts = io.tile([128, OHC], dt.bfloat16)
            S.dma_start(out=wcs[:, :], in_=wc[:, :])
            A.dma_start(out=rts[:, :], in_=rt[:, :])

            # ---- Phase A: K=64 tap GEMM over the 32-row window, two
            # 16-row groups stacked on psum partition halves.
            feedt = io.tile([2 * C, GFLAT], dt.bfloat16)
            S.dma_start(out=feedt[0:C, :], in_=xp[:, :])
            A.dma_start(out=feedt[C:2 * C, :], in_=yp[:, :])
            # staging: [0:27]=Xt g0, [32:59]=Yt g0, [64:91]=Xt g1, [96:123]=Yt g1
            st = stage.tile([128, RG * WB], dt.bfloat16)
            sv = st.rearrange("p (r b) -> p r b", b=WB)
            V.memset(sv[:, :, 0:3], 0.0)
            V.memset(sv[:, :, 121:124], 0.0)
            RCHUNK = [(0, 4), (4, 4), (8, 4), (12, 4)]
            psq = [psA.tile([128, 472], dt.float32, name=f"psq{i}",
                            tag=f"ps{i}") for i in range(4)]
            for g in range(2):
                for ci, (r0, nr) in enumerate(RCHUNK):
                    nn = nr * WP
                    off = (g * RG + r0) * WP
                    nc.tensor.matmul(
                        psq[ci][g * 64:(g + 1) * 64, :nn],
                        wcs[:, :], feedt[:, off:off + nn],
                        start=True, stop=True, tile_position=(0, g * 64))
            for ci, (r0, nr) in enumerate(RCHUNK):
                cdst = sv[:, r0:r0 + nr, 3:121]
                csrc = psq[ci][:, :nr * WP].rearrange("p (r w) -> p r w", w=WP)
                if ci % 2 == 0:
                    A.activation(cdst, csrc, ActF.Copy)
                else:
                    V.tensor_copy(out=cdst, in_=csrc)
            # write-out: h-major scatter
            for dram, base, rng in ((xtd, 0, S), (ytd, 32, A)):
                vd = dram.rearrange("h (t b) -> t h b", b=WB)
                for g in range(2):
                    rng.dma_start(
                        out=vd[:, g * RG:(g + 1) * RG, :],
                        in_=st[base + g * 64:base + g * 64 + NTAP, :]
                        .rearrange("p (r b) -> p r b", b=WB))

            # ---- read-back + kh-shifted copies
            xtT = stage.tile([GR, TWB], dt.bfloat16)
            ytT = stage.tile([GR, TWB], dt.bfloat16)
            S.dma_start(out=xtT[:, :], in_=xtd[:, :])
            A.dma_start(out=ytT[:, :], in_=ytd[:, :])
            XKH = [xtT]
            YKH = [ytT]
            for kh in (1, 2):
                xk = stage.tile([GR - 2, TWB], dt.bfloat16, name=f"xk{kh}")
                yk = stage.tile([GR - 2, TWB], dt.bfloat16, name=f"yk{kh}")
                S.dma_start(out=xk[0:GR - 2, :], in_=xtd[kh:kh + GR - 2, :])
                A.dma_start(out=yk[0:GR - 2, :], in_=ytd[kh:kh + GR - 2, :])
                XKH.append(xk)
                YKH.append(yk)

            # ---- Phase B images into one packed tile [HC, IMW] (bf16),
            # then broadcast to the 4 d-group partition bases.
            # blocks (cols): SAll 9*116 | RAll 6*150 | Fi,F0,F32,Gm1,G0,G1
            # 6*116 | Ri 160 | Rcorr 160 | R0,R32 2*150 | Gcol0 2 | Gcol32 4
            # | Rc0 1 | Rc32 1
            O_SA = 0
            O_RA = O_SA + 9 * W          # 1044
            O_F = O_RA + 6 * 150         # 1944
            O_RI = O_F + 6 * W           # 2640
            O_RC = O_RI + 160            # 2800
            O_R0 = O_RC + 160            # 2960
            O_R32 = O_R0 + 150           # 3110
            O_GC0 = O_R32 + 150          # 3260
            O_GC32 = O_GC0 + 2           # 3262
            O_RC0 = O_GC32 + 4           # 3266
            O_RC32 = O_RC0 + 1           # 3267
            IMW = O_RC32 + 1             # 3268
            IM = stage.tile([HC, IMW], dt.bfloat16)
            IM4 = stage.tile([128, IMW], dt.bfloat16)

            sall = strided(IM[:, :], O_SA, [(3 * W, 3), (W, 3), (1, W)])
            # left accumulates: term (kh,kw) -> last (kw+1) j-blocks
            V.memset(IM[:, O_SA:O_SA + 9 * W], 0.0)
            for kh in range(3):
                for kw in range(3):
                    t0 = kh * 3 + kw
                    src_ap = strided(XKH[kh][0:HC, :], t0 * WB + kw + 3,
                                     [(9 * WB, 3), (1, W)])
                    for j in range(2 - kw, 3):
                        dst = strided(IM[:, :], O_SA + j * 3 * W,
                                      [(W, 3), (1, W)])
                        V.tensor_tensor(out=dst, in0=dst, in1=src_ap,
                                        op=Alu.add)

            def sa(j, k):   # S_k[j] block [HC, W]; j index: 0=SA2,1=SA1,2=SA0
                jj = {0: 2, 1: 1, 2: 0}[j]
                return IM[:, O_SA + jj * 3 * W + k * W:
                          O_SA + jj * 3 * W + (k + 1) * W]

            # right accumulates: RAll = [RKc(3 kd) | RCc(3 kd)] of 150 cols,
            # data at col u+32, u in [-2,115] -> 30..148
            G_.memset(IM[:, O_RA:O_RA + 6 * 150], 0.0)
            for kh in range(3):
                for kw in range(3):
                    t0 = kh * 3 + kw
                    src_ap = strided(YKH[kh][0:HC, :], t0 * WB + kw + 2,
                                     [(9 * WB - 1, 3), (1, 118)])
                    for j in range(0, 2 if kw == 2 else 1):
                        dst = strided(IM[:, :], O_RA + j * 3 * 150 + 30,
                                      [(150, 3), (1, 118)])
                        G_.tensor_tensor(out=dst, in0=dst, in1=src_ap,
                                        op=Alu.add)

            def rkc(k):
                return IM[:, O_RA + k * 150:O_RA + (k + 1) * 150]

            def rcc(k, c0, c1):
                return IM[:, O_RA + (3 + k) * 150 + c0:
                          O_RA + (3 + k) * 150 + c1]

            FiB = IM[:, O_F + 0 * W:O_F + 1 * W]
            F0B = IM[:, O_F + 1 * W:O_F + 2 * W]
            F32B = IM[:, O_F + 2 * W:O_F + 3 * W]
            Gm1B = IM[:, O_F + 3 * W:O_F + 4 * W]
            G0B = IM[:, O_F + 4 * W:O_F + 5 * W]
            G1B = IM[:, O_F + 5 * W:O_F + 6 * W]
            V.tensor_tensor(out=F32B, in0=sa(0, 0), in1=sa(0, 1), op=Alu.add)
            V.tensor_tensor(out=F0B, in0=sa(0, 1), in1=sa(0, 2), op=Alu.add)
            V.tensor_tensor(out=FiB, in0=F32B, in1=sa(0, 2), op=Alu.add)
            V.tensor_tensor(out=Gm1B, in0=sa(1, 0), in1=sa(2, 1), op=Alu.add)
            V.tensor_tensor(out=G0B, in0=sa(0, 0), in1=sa(1, 1), op=Alu.add)
            V.tensor_tensor(out=G0B, in0=G0B, in1=sa(2, 2), op=Alu.add)
            V.tensor_tensor(out=G1B, in0=F32B, in1=sa(1, 2), op=Alu.add)
            # Gm2 alias = sa(0, 0); P20 = sa(0, 2); P21 = sa(1, 2)
            V.tensor_tensor(out=IM[:, O_GC0:O_GC0 + 1], in0=G0B[:, 0:1],
                            in1=sa(0, 0)[:, 0:1], op=Alu.subtract)
            V.tensor_tensor(out=IM[:, O_GC0 + 1:O_GC0 + 2], in0=G1B[:, 1:2],
                            in1=sa(0, 0)[:, 1:2], op=Alu.subtract)
            A.activation(IM[:, O_GC32:O_GC32 + 1], sa(2, 0)[:, 30:31],
                         ActF.Copy)
            A.activation(IM[:, O_GC32 + 1:O_GC32 + 2], Gm1B[:, 31:32],
                         ActF.Copy)
            V.tensor_tensor(out=IM[:, O_GC32 + 2:O_GC32 + 3],
                            in0=G0B[:, 32:33], in1=sa(2, 2)[:, 32:33],
                            op=Alu.subtract)
            V.tensor_tensor(out=IM[:, O_GC32 + 3:O_GC32 + 4],
                            in0=G1B[:, 33:34], in1=sa(1, 2)[:, 33:34],
                            op=Alu.subtract)
            # right folds; Ri/Rcorr live at col offset 42 (width 160)
            RiB = IM[:, O_RI:O_RI + 160]
            RcB = IM[:, O_RC:O_RC + 160]
            R0B = IM[:, O_R0:O_R0 + 150]
            R32B = IM[:, O_R32:O_R32 + 150]
            V.memset(RiB[:, 0:10], 0.0)
            V.memset(RcB[:, 0:10], 0.0)
            V.tensor_tensor(out=R32B, in0=rkc(0), in1=rkc(1), op=Alu.add)
            V.tensor_tensor(out=R0B, in0=rkc(1), in1=rkc(2), op=Alu.add)
            V.tensor_tensor(out=RiB[:, 10:160], in0=R32B, in1=rkc(2),
                            op=Alu.add)
            V.tensor_tensor(out=RcB[:, 10:160], in0=rcc(0, 0, 150),
                            in1=rcc(1, 0, 150), op=Alu.add)
            V.tensor_tensor(out=RcB[:, 10:160], in0=RcB[:, 10:160],
                            in1=rcc(2, 0, 150), op=Alu.add)
            V.tensor_tensor(out=IM[:, O_RC0:O_RC0 + 1],
                            in0=rcc(1, 147, 148), in1=rcc(2, 147, 148),
                            op=Alu.add)
            V.tensor_tensor(out=IM[:, O_RC32:O_RC32 + 1],
                            in0=rcc(0, 115, 116), in1=rcc(1, 115, 116),
                            op=Alu.add)
            # broadcast images to the 4 d-group partition bases
            for dg in range(4):
                rng = S if dg % 2 == 0 else A
                rng.dma_start(out=IM4[dg * 32:dg * 32 + HC, :],
                              in_=IM[:, :])

            def img4(dg, off, c0, c1):
                return IM4[dg * 32:dg * 32 + HC, off + c0:off + c1]

            # ---- Assembly: cost [128, 9*116] bf16, d = 9*dg + jd
            cost = stage.tile([128, 9 * W], dt.bfloat16)
            G_.memset(cost[:, :], 0.0)
            for dg in range(4):
                base = dg * 32
                cslab = cost[base:base + HC, :]
                cv = cslab.rearrange("p (j w) -> p j w", w=W)
                # 1. F select: keep where w - (9dg + jd) - 2 >= 0
                G_.affine_select(
                    out=cv[:, :, :],
                    in_=img4(dg, O_F, 0, W).unsqueeze(1)
                    .broadcast_to((HC, 9, W)),
                    pattern=[[-1, 9], [1, W]], base=-2 - 9 * dg,
                    compare_op=Alu.is_ge, fill=0.0, channel_multiplier=0)
                # 2. G diagonals (interior d in [dlo,31])
                for tp, dlo, go in ((-2, 2, 0), (-1, 1, 3), (0, 1, 4),
                                    (1, 1, 5)):
                    jlo = max(0, dlo - 9 * dg)
                    jhi = min(8, 31 - 9 * dg)
                    if jlo > jhi:
                        continue
                    cnt = jhi - jlo + 1
                    dst = strided(cslab, 117 * jlo + 9 * dg + tp,
                                  [(117, cnt)])
                    # go==0 -> Gm2 alias = S[2] kd0 block at O_SA + 0
                    if go == 0:
                        gsrc = img4(dg, O_SA, 0, W)
                    else:
                        gsrc = img4(dg, O_F, go * W, (go + 1) * W)
                    src = strided(gsrc, 9 * dg + jlo + tp, [(1, cnt)])
                    A.activation(dst, src, ActF.Copy)
                # 3. R diagonal add: cost[.,jd,w] += Ri[., 42 + w - 9dg - jd]
                V.tensor_tensor(
                    out=cv[:, :, :], in0=cv[:, :, :],
                    in1=strided(img4(dg, O_RI, 0, 160), 42 - 9 * dg,
                                [(-1, 9), (1, W)]),
                    op=Alu.add)
                # 4. right-edge corr (interior d): -= Rcorr[., 157-9dg-jd]
                jlo = max(0, 1 - 9 * dg)
                jhi = min(8, 31 - 9 * dg)
                if jlo <= jhi:
                    cnt = jhi - jlo + 1
                    dst = strided(cslab, 116 * jlo + 115, [(116, cnt)])
                    src = strided(img4(dg, O_RC, 0, 160),
                                  157 - 9 * dg - jlo, [(-1, cnt)])
                    V.tensor_tensor(out=dst, in0=dst, in1=src,
                                    op=Alu.subtract)
            # 5. fixup d=0 (dg0, jd0)
            c0v = cost[0:HC, 0:W]
            G_.affine_select(out=c0v, in_=img4(0, O_F, W, 2 * W),
                             pattern=[[1, W]], base=-2,
                             compare_op=Alu.is_ge, fill=0.0,
                             channel_multiplier=0)
            V.tensor_copy(out=cost[0:HC, 0:2], in_=img4(0, O_GC0, 0, 2))
            V.tensor_tensor(out=c0v, in0=c0v, in1=img4(0, O_R0, 32, 148),
                            op=Alu.add)
            V.tensor_tensor(out=cost[0:HC, 115:116],
                            in0=cost[0:HC, 115:116],
                            in1=img4(0, O_RC0, 0, 1), op=Alu.subtract)
            # 6. fixup d=32 (dg3, jd5)
            c32 = cost[96:96 + HC, 5 * W:6 * W]
            G_.affine_select(out=c32, in_=img4(3, O_F, 2 * W, 3 * W),
                             pattern=[[1, W]], base=-34,
                             compare_op=Alu.is_ge, fill=0.0,
                             channel_multiplier=0)
            V.tensor_copy(out=cost[96:96 + HC, 5 * W + 30:5 * W + 34],
                          in_=img4(3, O_GC32, 0, 4))
            V.tensor_tensor(out=c32, in0=c32, in1=img4(3, O_R32, 0, W),
                            op=Alu.add)
            V.tensor_tensor(out=cost[96:96 + HC, 5 * W + 115:5 * W + 116],
                            in0=cost[96:96 + HC, 5 * W + 115:5 * W + 116],
                            in1=img4(3, O_RC32, 0, 1), op=Alu.subtract)

            # ---- Resize: per-dgroup row matmul (K=32, zero-padded rows),
            # column phases contiguous, split in two dg-halves for DMA
            # overlap. u/v/p layout: [65, 36*116] (d = 9dg + jd).
            u = stage.tile([OHC, DP * W], dt.bfloat16)
            v = stage.tile([OHC, DP * W], dt.bfloat16)
            p0 = stage.tile([OHC, DP * W], dt.float32)
            p1 = stage.tile([OHC, DP * W], dt.bfloat16)
            p2 = stage.tile([OHC, DP * W], dt.bfloat16)
            uvv = u.rearrange("p (d w) -> p d w", w=W)
            vvv = v.rearrange("p (d w) -> p d w", w=W)
            p1v = p1.rearrange("p (d w) -> p d w", w=W)
            p2v = p2.rearrange("p (d w) -> p d w", w=W)
            for dg in range(4):
                nd = 9 if dg < 3 else 6
                chunks = ([(0, 348), (348, 348), (696, 348)] if dg < 3
                          else [(0, 348), (348, 348)])
                for off, nn in chunks:
                    ps = psR.tile([OHC, 348], dt.float32, tag="psR")
                    nc.tensor.matmul(
                        ps[:, :nn], rts[dg * 32:(dg + 1) * 32, :],
                        cost[dg * 32:(dg + 1) * 32, off:off + nn],
                        start=True, stop=True,
                        tile_position=(dg * 32, 0))
                    uo = dg * 9 * W + off
                    A.activation(u[:, uo:uo + nn], ps[:, :nn], ActF.Copy,
                                 scale=1.0 / 3.0)
                    A.activation(p0[:, uo:uo + nn], ps[:, :nn], ActF.Copy)
                c0 = dg * 9 * W
                dlo, dhi = dg * 9, dg * 9 + nd
                V.tensor_scalar_mul(out=v[:, c0:c0 + nd * W],
                                    in0=u[:, c0:c0 + nd * W], scalar1=2.0)
                V.tensor_tensor(out=p1v[:, dlo:dhi, 0:115],
                                in0=vvv[:, dlo:dhi, 0:115],
                                in1=uvv[:, dlo:dhi, 1:116], op=Alu.add)
                V.tensor_tensor(out=p2v[:, dlo:dhi, 0:115],
                                in0=uvv[:, dlo:dhi, 0:115],
                                in1=vvv[:, dlo:dhi, 1:116], op=Alu.add)
                if dg % 2 == 1:
                    hlo, hhi = (dg - 1) * 9, dg * 9 + nd
                    S.dma_start(
                        out=out0[hlo:hhi].rearrange("d j w -> j d w"),
                        in_=p0.rearrange(
                            "p (d w) -> p d w", w=W)[:, hlo:hhi, :])
                    A.dma_start(
                        out=out12[0, hlo:hhi].rearrange(
                            "d j w -> j d w")[:, :, 0:W - 1],
                        in_=p1.rearrange(
                            "p (d w) -> p d w", w=W)[:, hlo:hhi, 0:W - 1])
                    S.dma_start(
                        out=out12[1, hlo:hhi].rearrange(
                            "d j w -> j d w")[:, :, 0:W - 1],
                        in_=p2.rearrange(
                            "p (d w) -> p d w", w=W)[:, hlo:hhi, 0:W - 1])
    nc.compile()
    return nc


def _prep_inputs(x_feat, y_feat, w_match):
    """Host-side shard prep: per-core input dicts."""
    x_feat = np.asarray(x_feat, dtype=np.float32)
    y_feat = np.asarray(y_feat, dtype=np.float32)
    w_match = np.asarray(w_match, dtype=np.float32)
    wcm = np.zeros((2 * C, 64), dtype=BF16)
    wcm[:C, :NTAP] = w_match[0, :C].reshape(C, NTAP)
    wcm[C:, 32:32 + NTAP] = w_match[0, C:].reshape(C, NTAP)
    Rt = _row_matrix()
    in_maps = []
    for core in range(8):
        n, q = divmod(core, 4)
        h0 = H0S[q]
        xg = np.zeros((C, 92, WP), dtype=BF16)
        yg = np.zeros((C, 92, WP), dtype=BF16)
        xg[:, 1:89, 1:117] = x_feat[n, 2]
        yg[:, 1:89, 1:117] = y_feat[n, 2]
        xw = np.zeros((C, GR, WP), dtype=BF16)
        yw = np.zeros((C, GR, WP), dtype=BF16)
        nrow = min(GR, 92 - h0)
        xw[:, :nrow] = xg[:, h0:h0 + nrow]
        yw[:, :nrow] = yg[:, h0:h0 + nrow]
        rtc = np.zeros((128, OHC), dtype=np.float32)
        for r in range(HC):
            hh = h0 + r
            if hh < H:
                for dg in range(4):
                    rtc[dg * 32 + r] = Rt[hh, q * OHC:(q + 1) * OHC]
        in_maps.append({
            "xp": xw.reshape(C, GFLAT),
            "yp": yw.reshape(C, GFLAT),
            "wc": wcm,
            "rt": rtc.astype(BF16),
        })
    return in_maps


def _interleave(out_slice, p0, p12):
    """p0 [33,65,116] f32, p12 [2,33,65,116] bf16 -> out_slice [33,65,346]."""
    out_slice[:, :, 0::3] = p0
    out_slice[:, :, 1::3] = p12[0, :, :, :115].astype(np.float32)
    out_slice[:, :, 2::3] = p12[1, :, :, :115].astype(np.float32)


def kernel(x_feat, y_feat, w_match):
    from concourse.bass_utils import run_bass_kernel_spmd

    if "nc" not in _BUILT:
        _BUILT["nc"] = _build_nc()
    nc = _BUILT["nc"]
    in_maps = _prep_inputs(x_feat, y_feat, w_match)
    trace = bool(int(os.environ.get("KERNEL_TRACE", "0")))
    res = run_bass_kernel_spmd(
        nc, in_maps, core_ids=list(range(8)),
        trace=trace,
        trace_cores=list(range(8)) if trace else None,
    )
    _BUILT["last_result"] = res
    out = np.empty((2, D, OH, OW), dtype=np.float32)
    for core in range(8):
        n, q = divmod(core, 4)
        _interleave(out[n, :, q * OHC:(q + 1) * OHC, :],
                    res.results[core]["out0"], res.results[core]["out12"])
    return out
